# revision 1
# baseline (speedup 1.0000x reference)
"""Trainium2 Bass kernel for nn_ChannelAttentionModule (cyclic window mean +
channel attention). Self-contained: accepts FULL inputs, shards spatial dim
across 8 NeuronCores, returns FULL [64, 256] output.

Math: cyclic_window_mean over the batch axis is a matmul with a fixed [64,64]
window matrix M.  Per core (spatial shard of 512 of the 4096 positions):
  - stream x as [128, 2048] tiles (partitions = 2 spatial half-groups x 64 batch)
  - PE: y = blockdiag(M^T, M^T)^T @ x_tile  -> windowed means per position
  - DVE: running elementwise max over tiles    (-> spatial max of y)
  - PE: accumulate vstack(M^T, M^T)^T @ x_tile into one PSUM bank
        (-> spatial sum of y, partition halves pre-folded)
  - AllGather the packed [64, 512] (max | sum) partials across 8 cores
  - every core folds + computes the tiny MLP / softmax / final window matmul
"""

import os
import sys

import numpy as np

for _p in ("/opt/trn_rl_repo", "/root/.axon_site/_ro/trn_rl_repo"):
    if os.path.isdir(_p) and _p not in sys.path:
        sys.path.insert(0, _p)

import concourse.bass as bass
import concourse.mybir as mybir
import concourse.tile as tile
from concourse import bacc
from concourse import bass_utils as _bu
from concourse.bass_utils import run_bass_kernel_spmd

# Redundant-LDWEIGHTS elision: every streaming matmul reuses the same
# stationary weights, so let walrus's ldw-opt pass drop the reloads.
_orig_run_command = _bu.run_command

def _run_command_ldwopt(argv, **kwargs):
    argv = [a.replace("--enable-ldw-opt=false", "--enable-ldw-opt=true")
            if isinstance(a, str) else a for a in argv]
    return _orig_run_command(argv, **kwargs)

# ldw-opt breaks bf16 ldweights lowering; leave disabled
# _bu.run_command = _run_command_ldwopt

B = 64          # batch
S = 64 * 64     # flattened spatial
C = 256         # channels
CE = 768        # hidden (C * 3)
NCORES = 8
S_CORE = S // NCORES   # 512 spatial positions per core
G = 32                 # positions per partition half-group per tile
TP = 2 * G             # 16 positions per tile
NT = S_CORE // TP      # 32 tiles per core
F = G * C              # 2048 free elements per tile
FQ = 512               # matmul moving free dim (one PSUM bank, fp32)
NQ = F // FQ           # 4 matmul chunks per tile
DT = mybir.dt.float32
AF = mybir.ActivationFunctionType


def _win_matrix(w: int) -> np.ndarray:
    """M such that cyclic_window_mean(x, w) == M @ x (along axis 0)."""
    m = np.zeros((B, B), np.float64)
    for i in range(B):
        if i >= w:
            m[i, i - w:i] = 1.0 / w
        else:
            m[i, : i + 1] = 1.0 / (w + 1)
            m[i, B - (w - i):] = 1.0 / (w + 1)
    return m.astype(np.float32)


def _build_program(wn: int) -> bass.Bass:
    nc = bacc.Bacc(
        "TRN2", target_bir_lowering=False, debug=False, num_devices=NCORES
    )

    DTR = mybir.dt.float32r  # fp32 bits, single-pass PE mode (1 cyc/row)
    DTB = mybir.dt.bfloat16
    xs = nc.declare_dram_parameter("xs", [B, S_CORE, C], DTB, isOutput=False)
    wblk_d = nc.declare_dram_parameter("wblk", [128, 128], DTB, isOutput=False)
    wstk_d = nc.declare_dram_parameter("wstk", [128, 64], DTB, isOutput=False)
    wsc_d = nc.declare_dram_parameter("wsc", [64, 1], DT, isOutput=False)
    wfin_d = nc.declare_dram_parameter("wfin", [64, 64], DTR, isOutput=False)
    eye_d = nc.declare_dram_parameter("eye64", [64, 64], DT, isOutput=False)
    w1m_d = nc.declare_dram_parameter("w1m", [C, CE], DTR, isOutput=False)
    b1m_d = nc.declare_dram_parameter("b1m", [CE], DT, isOutput=False)
    w2m_d = nc.declare_dram_parameter("w2m", [CE, C], DTR, isOutput=False)
    b2m_d = nc.declare_dram_parameter("b2m", [1, C], DT, isOutput=False)
    w1a_d = nc.declare_dram_parameter("w1a", [C, CE], DTR, isOutput=False)
    b1a_d = nc.declare_dram_parameter("b1a", [CE], DT, isOutput=False)
    w2a_d = nc.declare_dram_parameter("w2a", [CE, C], DTR, isOutput=False)
    b2a_d = nc.declare_dram_parameter("b2a", [1, C], DT, isOutput=False)
    out_d = nc.declare_dram_parameter("out", [B, C], DT, isOutput=True)

    with tile.TileContext(nc) as tc:
        with (
            tc.tile_pool(name="const", bufs=1) as cpool,
            tc.tile_pool(name="x", bufs=5) as xpool,
            tc.tile_pool(name="pmax", bufs=2, space="PSUM") as pmax,
            tc.tile_pool(name="psum_sum", bufs=1, space="PSUM") as psump,
            tc.tile_pool(name="ptail", bufs=2, space="PSUM") as ptail,
            tc.tile_pool(name="pa_pool", bufs=1, space="PSUM") as papool,
            tc.tile_pool(name="sb", bufs=1) as spool,
            tc.tile_pool(name="dram", bufs=1, space="DRAM") as dpool,
        ):
            # ---- constants into SBUF (all 128-partition tiles: matmul
            #      operands must sit at base partition 0) ----
            wblk_sb = cpool.tile([128, 128], DTB, tag="wblk")
            nc.sync.dma_start(wblk_sb[:], wblk_d[:])
            wstk_sb = cpool.tile([128, 64], DTB, tag="wstk")
            nc.sync.dma_start(wstk_sb[:], wstk_d[:])
            wfin_sb = cpool.tile([128, 64], DTR, tag="wfin")
            nc.gpsimd.dma_start(wfin_sb[0:64, :], wfin_d[:])
            wsc_sb = cpool.tile([128, 1], DT, tag="wsc")
            nc.gpsimd.dma_start(wsc_sb[0:64, :], wsc_d[:])
            eye_sb = cpool.tile([128, 64], DT, tag="eye")
            nc.gpsimd.dma_start(eye_sb[0:64, :], eye_d[:])
            # W1 as [128, 2, CE]: partition = k-within-half, free = (half, n)
            w1m_sb = cpool.tile([128, 2, CE], DTR, tag="w1m")
            nc.gpsimd.dma_start(w1m_sb[:], w1m_d[:].rearrange("(h k) n -> k h n", h=2))
            w1a_sb = cpool.tile([128, 2, CE], DTR, tag="w1a")
            nc.gpsimd.dma_start(w1a_sb[:], w1a_d[:].rearrange("(h k) n -> k h n", h=2))
            # W2 as [128, 6, C]: partition = k-within-chunk, free = (chunk, n)
            w2m_sb = cpool.tile([128, 6, C], DTR, tag="w2m")
            nc.gpsimd.dma_start(w2m_sb[:], w2m_d[:].rearrange("(m k) n -> k m n", m=6))
            w2a_sb = cpool.tile([128, 6, C], DTR, tag="w2a")
            nc.gpsimd.dma_start(w2a_sb[:], w2a_d[:].rearrange("(m k) n -> k m n", m=6))
            # b1 transposed to [128, 6] -> per-partition bias for the ACT relu
            b1m_sb = cpool.tile([128, 6], DT, tag="b1m")
            nc.gpsimd.dma_start(b1m_sb[:], b1m_d[:].rearrange("(m k) -> k m", m=6))
            b1a_sb = cpool.tile([128, 6], DT, tag="b1a")
            nc.gpsimd.dma_start(b1a_sb[:], b1a_d[:].rearrange("(m k) -> k m", m=6))
            # row-0 smalls: ones[0:64] | b2m[64:320] | b2a[320:576]
            smalls = cpool.tile([128, 576], DT, tag="smalls")
            nc.vector.memset(smalls[0:1, 0:64], 1.0)
            nc.gpsimd.dma_start(smalls[0:1, 64:320], b2m_d[:])
            nc.gpsimd.dma_start(smalls[0:1, 320:576], b2a_d[:])
            ones_sb = smalls[0:1, 0:64]
            b2m_sb = smalls[0:1, 64:320]
            b2a_sb = smalls[0:1, 320:576]

            # ---- main streaming phase ----
            # Partition packing p = 2*b + h (h = spatial half-group INNER):
            # one DMA per tile covers all 128 partitions with a flat outer
            # batch dim, so descriptors spread over all 16 SDMA engines
            # (engine = partition/8; a 64-partition half engages only 8).
            max_acc = spool.tile([128, 2 * FQ], DTB, tag="max_acc")
            nc.vector.memset(max_acc[:], -1e30)
            sum_ps = psump.tile([128, FQ], DT, tag="sum_ps")
            wblk_r = wblk_sb[:]
            wstk_r = wstk_sb[:]

            # s_local = t*TP + h*G + g ; partition = (b, h) ; free = (g, c)
            xs_r = xs[:].rearrange("b (t h g) c -> t b h (g c)", h=2, g=G)
            for t in range(NT):
                xt = xpool.tile([128, F], DTB, tag="xt")
                nc.sync.dma_start(xt[:], xs_r[t])
                xt_r = xt[:]
                for half in range(F // (2 * FQ)):
                    ym = pmax.tile([128, 2 * FQ], DT, tag="ym")
                    for q in range(2):
                        col = (2 * half + q) * FQ
                        nc.tensor.matmul(
                            ym[:, q * FQ:(q + 1) * FQ], wblk_r,
                            xt_r[:, col:col + FQ], start=True, stop=True,
                        )
                    # evict to bf16 SBUF on the (idle) ACT engine so the
                    # DVE max chain runs at 2x_1p instead of 1x-from-PSUM
                    ymb = spool.tile([128, 2 * FQ], DTB, tag="ymb", bufs=4)
                    nc.scalar.copy(ymb[:], ym[:])
                    nc.vector.tensor_max(max_acc[:], max_acc[:], ymb[:])
                    for q in range(2):
                        col = (2 * half + q) * FQ
                        first = (t == 0 and half == 0 and q == 0)
                        last = (t == NT - 1
                                and half == F // (2 * FQ) - 1 and q == 1)
                        nc.tensor.matmul(
                            sum_ps[0:64, :], wstk_r, xt_r[:, col:col + FQ],
                            start=first, stop=last, skip_group_check=True,
                        )

            # ---- fold local partials to [64, C] each, pack as [64, 2C] ----
            # partition fold pairs (2i, 2i+1): extract strided halves via DMA
            ev64 = spool.tile([128, 2 * FQ], DTB, tag="ev64")
            od64 = spool.tile([128, 2 * FQ], DTB, tag="od64")
            nc.sync.dma_start(ev64[0:64, :], max_acc[0:128:2, :])
            nc.scalar.dma_start(od64[0:64, :], max_acc[1:128:2, :])
            u = spool.tile([128, 2 * FQ], DTB, tag="u")
            nc.vector.tensor_max(u[0:64, :], ev64[0:64, :], od64[0:64, :])
            nc.vector.tensor_max(u[0:64, 0:FQ], u[0:64, 0:FQ], u[0:64, FQ:2 * FQ])
            pk = spool.tile([128, 2 * C], DT, tag="pk")
            nc.vector.tensor_max(pk[0:64, 0:C], u[0:64, 0:C], u[0:64, C:2 * C])
            su = spool.tile([128, FQ], DT, tag="su")
            nc.scalar.copy(su[0:64, :], sum_ps[0:64, :])
            nc.vector.tensor_add(
                pk[0:64, C:2 * C], su[0:64, 0:C], su[0:64, C:2 * C]
            )

            # ---- cross-core combine: one AllGather + local fold ----
            rg = [list(range(NCORES))]
            gin = dpool.tile([64, 2 * C], DT, tag="gin")
            gout = dpool.tile([NCORES * 64, 2 * C], DT, tag="gout")
            nc.sync.dma_start(gin[:], pk[0:64, :])
            nc.gpsimd.collective_compute(
                "AllGather", mybir.AluOpType.bypass, replica_groups=rg,
                ins=[gin.opt()], outs=[gout.opt()],
            )
            g_sb = spool.tile([128, NCORES, 2 * C], DT, tag="g_sb")
            nc.scalar.dma_start(
                g_sb[0:64, :, :], gout[:].rearrange("(r b) n -> b r n", r=NCORES)
            )
            mxf = spool.tile([128, C], DT, tag="mxf")
            svf = spool.tile([128, C], DT, tag="svf")
            nc.vector.tensor_copy(mxf[0:64, :], g_sb[0:64, 0, 0:C])
            nc.vector.tensor_copy(svf[0:64, :], g_sb[0:64, 0, C:2 * C])
            for r in range(1, NCORES):
                nc.vector.tensor_max(mxf[0:64, :], mxf[0:64, :], g_sb[0:64, r, 0:C])
                nc.vector.tensor_add(
                    svf[0:64, :], svf[0:64, :], g_sb[0:64, r, C:2 * C]
                )

            # window sums -> window means: scale rows by 1/w_i
            nc.vector.tensor_scalar_mul(mxf[0:64, :], mxf[0:64, :], wsc_sb[0:64, :])
            nc.vector.tensor_scalar_mul(svf[0:64, :], svf[0:64, :], wsc_sb[0:64, :])

            # ---- transpose [64, 256] -> [128, 2, 64] (chunked over C) ----
            def transpose_bc(src, tag):
                dst = spool.tile([128, 2, 64], DTR, tag=tag)
                for ch in range(2):
                    pt = ptail.tile([128, 64], DT, tag="ph")
                    nc.tensor.transpose(
                        pt[:], src[0:64, ch * 128:(ch + 1) * 128], eye_sb[0:64, :]
                    )
                    nc.scalar.copy(dst[:, ch, :], pt[:])
                return dst

            mxT = transpose_bc(mxf, "mxT")
            svT = transpose_bc(svf, "svT")

            # ---- the two tiny MLPs (relu(relu(v @ W1 + b1) @ W2 + b2)) ----
            def mlp(vT, w1_sb, b1_sb, w2_sb, b2_sb, tag):
                h1 = spool.tile([128, 6, 64], DTR, tag=f"h1_{tag}")
                for m in range(6):
                    ph = ptail.tile([128, 64], DT, tag="ph")
                    nc.tensor.matmul(
                        ph[:], w1_sb[:, 0, m * 128:(m + 1) * 128], vT[:, 0, :],
                        start=True, stop=False,
                    )
                    nc.tensor.matmul(
                        ph[:], w1_sb[:, 1, m * 128:(m + 1) * 128], vT[:, 1, :],
                        start=False, stop=True,
                    )
                    nc.scalar.activation(
                        h1[:, m, :], ph[:], AF.Relu, bias=b1_sb[:, m:m + 1]
                    )
                pa = papool.tile([128, C], DT, tag="pa")
                for m in range(6):
                    nc.tensor.matmul(
                        pa[0:64, :], h1[:, m, :], w2_sb[:, m, :],
                        start=(m == 0), stop=False,
                    )
                nc.tensor.matmul(
                    pa[0:64, :], ones_sb, b2_sb, start=False, stop=True
                )
                dst = spool.tile([128, C], DT, tag=f"mlp_{tag}")
                nc.scalar.activation(dst[0:64, :], pa[0:64, :], AF.Relu)
                return dst

            m_sb = mlp(mxT, w1m_sb, b1m_sb, w2m_sb, b2m_sb, "m")
            a_sb = mlp(svT, w1a_sb, b1a_sb, w2a_sb, b2a_sb, "a")

            # ---- sigmoid(m + a), softmax over channels ----
            t_sb = spool.tile([128, C], DT, tag="t_sb")
            nc.vector.tensor_add(t_sb[0:64, :], m_sb[0:64, :], a_sb[0:64, :])
            s_sb = spool.tile([128, C], DT, tag="s_sb")
            nc.scalar.activation(s_sb[0:64, :], t_sb[0:64, :], AF.Sigmoid)
            red = spool.tile([128, 4], DT, tag="red")  # rsum | rinv
            e_sb = spool.tile([128, C], DT, tag="e_sb")
            nc.scalar.activation(e_sb[0:64, :], s_sb[0:64, :], AF.Exp)
            nc.vector.tensor_reduce(
                red[0:64, 1:2], e_sb[0:64, :], axis=mybir.AxisListType.X,
                op=mybir.AluOpType.add,
            )
            nc.vector.reciprocal(red[0:64, 2:3], red[0:64, 1:2])
            att = spool.tile([128, C], DTR, tag="att")
            nc.vector.tensor_scalar_mul(att[0:64, :], e_sb[0:64, :], red[0:64, 2:3])

            # ---- final cyclic window mean + store ----
            po = papool.tile([128, C], DT, tag="pa")
            nc.tensor.matmul(
                po[0:64, :], wfin_sb[0:64, :], att[0:64, :], start=True, stop=True
            )
            ob = spool.tile([128, C], DT, tag="ob")
            nc.scalar.copy(ob[0:64, :], po[0:64, :])
            nc.sync.dma_start(out_d[:], ob[0:64, :])

    return nc


def run(inputs: dict, trace: bool = False, tmpdir: str | None = None):
    """Returns (full_output [64,256] f32, exec_time_ns or None)."""
    wn = int(np.asarray(inputs["windows"]))
    x = np.ascontiguousarray(np.asarray(inputs["x"], np.float32)).reshape(B, S, C)

    import ml_dtypes
    mwin = _win_matrix(wn)
    mt = np.ascontiguousarray(mwin.T)
    # 0/1 window-membership matrix (exact in bf16); per-row counts -> wsc
    m01 = (mwin > 0).astype(np.float32).T           # [j, i]
    cnt = (mwin > 0).sum(axis=1).astype(np.float32)  # rows of M
    wsc = (1.0 / cnt).reshape(64, 1)
    # partition packing p = 2*b + h: wblk[2j+h, 2i+h'] = m01[j,i] iff h==h'
    wblk = np.zeros((128, 128), np.float32)
    wstk = np.zeros((128, 64), np.float32)
    for h in range(2):
        wblk[h::2, h::2] = m01
        wstk[h::2, :] = m01
    wblk = wblk.astype(ml_dtypes.bfloat16)
    wstk = np.ascontiguousarray(wstk).astype(ml_dtypes.bfloat16)
    eye = np.eye(64, dtype=np.float32)

    common = {
        "wblk": wblk,
        "wstk": wstk,
        "wsc": wsc.astype(np.float32),
        "wfin": mt,
        "eye64": eye,
        "w1m": np.asarray(inputs["W1_max"], np.float32),
        "b1m": np.asarray(inputs["b1_max"], np.float32).reshape(CE),
        "w2m": np.asarray(inputs["W2_max"], np.float32),
        "b2m": np.asarray(inputs["b2_max"], np.float32).reshape(1, C),
        # avg branch consumes the raw spatial SUM; fold the 1/S into W1_avg
        "w1a": np.asarray(inputs["W1_avg"], np.float32) / np.float32(S),
        "b1a": np.asarray(inputs["b1_avg"], np.float32).reshape(CE),
        "w2a": np.asarray(inputs["W2_avg"], np.float32),
        "b2a": np.asarray(inputs["b2_avg"], np.float32).reshape(1, C),
    }
    in_maps = []
    for k in range(NCORES):
        m = dict(common)
        m["xs"] = np.ascontiguousarray(x[:, k * S_CORE:(k + 1) * S_CORE, :]).astype(ml_dtypes.bfloat16)
        in_maps.append(m)

    nc = _build_program(wn)
    nc.compile()
    res = run_bass_kernel_spmd(
        nc, in_maps, list(range(NCORES)), trace=trace, tmpdir=tmpdir,
    )
    out = np.asarray(res.results[0]["out"], np.float32)
    return out, res.exec_time_ns


def kernel(**inputs) -> np.ndarray:
    out, _ = run(inputs, trace=False)
    return out



# revision 8
# speedup vs baseline: 1.0649x; 1.0649x over previous
"""Trainium2 Bass kernel for nn_ChannelAttentionModule (cyclic window mean +
channel attention). Self-contained: accepts FULL inputs, shards spatial dim
across 8 NeuronCores, returns FULL [64, 256] output.

Math: cyclic_window_mean over the batch axis is a matmul with a fixed [64,64]
window matrix M.  Per core (spatial shard of 512 of the 4096 positions):
  - stream x as [128, 8192] bf16 tiles (partition p = 2b + h, h = spatial
    half-group; free = (g, c) with g = 32 positions, c = 256 channels)
  - PE pass 1: blockdiag(M01,M01)^T-style stationary -> per-position window
    SUMS into PSUM duos [128, 1024]
  - drain: most duos ACT-evicted to bf16 SBUF then DVE max-accumulated in
    2x mode; some duos DVE-maxed straight from PSUM (engine balance)
  - PE pass 2 (85%): vstack(M01,M01) stationary accumulates the spatial SUM
    into one PSUM bank; the last ~15% of sum work runs as DVE bf16 adds on
    the already-resident x tiles
  - fold g/h locally -> [64, 256] max partial + [64, 256] sum partial,
    packed [64, 512] bf16 -> AllGather -> every core folds 8 ranks and
    computes the tiny MLP / softmax / final window matmul redundantly.
"""

import os
import sys

import numpy as np

for _p in ("/opt/trn_rl_repo", "/root/.axon_site/_ro/trn_rl_repo"):
    if os.path.isdir(_p) and _p not in sys.path:
        sys.path.insert(0, _p)

import concourse.bass as bass
import concourse.mybir as mybir
import concourse.tile as tile
from concourse import bacc
from concourse.bass_utils import run_bass_kernel_spmd

B = 64          # batch
S = 64 * 64     # flattened spatial
C = 256         # channels
CE = 768        # hidden (C * 3)
NCORES = 8
S_CORE = S // NCORES   # 512 spatial positions per core
G = 32                 # positions per partition half-group per tile
TP = 2 * G             # 64 positions per tile
NT = S_CORE // TP      # 8 tiles per core
F = G * C              # 8192 free elements per tile
FQ = 512               # matmul moving free dim (one PSUM bank, fp32)
ND = F // 1024         # 8 PSUM duos [128, 1024] per tile
DT = mybir.dt.float32
DTB = mybir.dt.bfloat16
AF = mybir.ActivationFunctionType


def _dve_direct(t: int, d: int) -> bool:
    """Duos the DVE maxes straight from PSUM (no ACT eviction)."""
    return d == 7 or (t % 2 == 1 and d == 6)


def _dve_sum(t: int, d: int) -> bool:
    """Duos whose spatial-sum contribution comes from DVE adds on xt."""
    return t == 7 or (t == 6 and d >= 6)


def _win_matrix(w: int) -> np.ndarray:
    """M such that cyclic_window_mean(x, w) == M @ x (along axis 0)."""
    m = np.zeros((B, B), np.float64)
    for i in range(B):
        if i >= w:
            m[i, i - w:i] = 1.0 / w
        else:
            m[i, : i + 1] = 1.0 / (w + 1)
            m[i, B - (w - i):] = 1.0 / (w + 1)
    return m.astype(np.float32)


def _build_program(wn: int) -> bass.Bass:
    nc = bacc.Bacc(
        "TRN2", target_bir_lowering=False, debug=False, num_devices=NCORES
    )

    xs = nc.declare_dram_parameter("xs", [B, S_CORE, C], DTB, isOutput=False)
    wblk_d = nc.declare_dram_parameter("wblk", [128, 128], DTB, isOutput=False)
    wstk_d = nc.declare_dram_parameter("wstk", [128, 64], DTB, isOutput=False)
    wsc_d = nc.declare_dram_parameter("wsc", [64, 1], DT, isOutput=False)
    wfin_d = nc.declare_dram_parameter("wfin", [64, 64], DTB, isOutput=False)
    eye_d = nc.declare_dram_parameter("eye64", [64, 64], DT, isOutput=False)
    w1m_d = nc.declare_dram_parameter("w1m", [C, CE], DTB, isOutput=False)
    b1m_d = nc.declare_dram_parameter("b1m", [CE], DT, isOutput=False)
    w2m_d = nc.declare_dram_parameter("w2m", [CE, C], DTB, isOutput=False)
    b2m_d = nc.declare_dram_parameter("b2m", [1, C], DT, isOutput=False)
    w1a_d = nc.declare_dram_parameter("w1a", [C, CE], DTB, isOutput=False)
    b1a_d = nc.declare_dram_parameter("b1a", [CE], DT, isOutput=False)
    w2a_d = nc.declare_dram_parameter("w2a", [CE, C], DTB, isOutput=False)
    b2a_d = nc.declare_dram_parameter("b2a", [1, C], DT, isOutput=False)
    out_d = nc.declare_dram_parameter("out", [B, C], DT, isOutput=True)

    with tile.TileContext(nc) as tc:
        with (
            tc.tile_pool(name="const", bufs=1) as cpool,
            tc.tile_pool(name="x", bufs=4) as xpool,
            tc.tile_pool(name="ymb", bufs=4) as ypool,
            tc.tile_pool(name="pduo", bufs=2, space="PSUM") as ppool,
            tc.tile_pool(name="psum_sum", bufs=1, space="PSUM") as psump,
            tc.tile_pool(name="ptail", bufs=2, space="PSUM") as ptail,
            tc.tile_pool(name="pa_pool", bufs=1, space="PSUM") as papool,
            tc.tile_pool(name="sb", bufs=1) as spool,
            tc.tile_pool(name="dram", bufs=1, space="DRAM") as dpool,
        ):
            rg = [list(range(NCORES))]

            # ---- warmup collective: sync the ranks early so the real
            #      AllGather later doesn't eat the launch/ramp skew ----
            wrm_in = dpool.tile([8, 16], DT, tag="wrm_in")
            wrm_out = dpool.tile([NCORES * 8, 16], DT, tag="wrm_out")
            wrm_sb = cpool.tile([8, 16], DT, tag="wrm_sb")
            nc.vector.memset(wrm_sb[:], 0.0)
            nc.sync.dma_start(wrm_in[:], wrm_sb[:])
            nc.gpsimd.collective_compute(
                "AllGather", mybir.AluOpType.bypass, replica_groups=rg,
                ins=[wrm_in.opt()], outs=[wrm_out.opt()],
            )

            # ---- constants into SBUF ----
            wblk_sb = cpool.tile([128, 128], DTB, tag="wblk")
            nc.sync.dma_start(wblk_sb[:], wblk_d[:])
            wstk_sb = cpool.tile([128, 64], DTB, tag="wstk")
            nc.sync.dma_start(wstk_sb[:], wstk_d[:])
            wfin_sb = cpool.tile([128, 64], DTB, tag="wfin")
            nc.gpsimd.dma_start(wfin_sb[0:64, :], wfin_d[:])
            wsc_sb = cpool.tile([128, 1], DT, tag="wsc")
            nc.gpsimd.dma_start(wsc_sb[0:64, :], wsc_d[:])
            eye_sb = cpool.tile([128, 64], DT, tag="eye")
            nc.gpsimd.dma_start(eye_sb[0:64, :], eye_d[:])
            # W1 as [128, 2, CE]: partition = k-within-half, free = (half, n)
            w1m_sb = cpool.tile([128, 2, CE], DTB, tag="w1m")
            nc.gpsimd.dma_start(w1m_sb[:], w1m_d[:].rearrange("(h k) n -> k h n", h=2))
            w1a_sb = cpool.tile([128, 2, CE], DTB, tag="w1a")
            nc.gpsimd.dma_start(w1a_sb[:], w1a_d[:].rearrange("(h k) n -> k h n", h=2))
            # W2 as [128, 6, C]: partition = k-within-chunk, free = (chunk, n)
            w2m_sb = cpool.tile([128, 6, C], DTB, tag="w2m")
            nc.gpsimd.dma_start(w2m_sb[:], w2m_d[:].rearrange("(m k) n -> k m n", m=6))
            w2a_sb = cpool.tile([128, 6, C], DTB, tag="w2a")
            nc.gpsimd.dma_start(w2a_sb[:], w2a_d[:].rearrange("(m k) n -> k m n", m=6))
            # b1 transposed to [128, 6] -> per-partition bias for the ACT relu
            b1m_sb = cpool.tile([128, 6], DT, tag="b1m")
            nc.gpsimd.dma_start(b1m_sb[:], b1m_d[:].rearrange("(m k) -> k m", m=6))
            b1a_sb = cpool.tile([128, 6], DT, tag="b1a")
            nc.gpsimd.dma_start(b1a_sb[:], b1a_d[:].rearrange("(m k) -> k m", m=6))
            # row-0 smalls: ones[0:64] | b2m[64:320] | b2a[320:576]
            smalls = cpool.tile([128, 576], DT, tag="smalls")
            nc.vector.memset(smalls[0:1, 0:64], 1.0)
            nc.gpsimd.dma_start(smalls[0:1, 64:320], b2m_d[:])
            nc.gpsimd.dma_start(smalls[0:1, 320:576], b2a_d[:])
            ones_sb = smalls[0:1, 0:64]
            b2m_sb = smalls[0:1, 64:320]
            b2a_sb = smalls[0:1, 320:576]

            # preload the exp_and_others ACT table set (exp + tanh + relu +
            # copy) while the first x tile streams, so no table switch ever
            # lands on the critical path
            scratch = cpool.tile([128, 16], DT, tag="scratch")
            nc.vector.memset(scratch[0:1, :], 0.0)
            nc.scalar.activation(scratch[0:1, :], scratch[0:1, :], AF.Exp)

            # ---- accumulators ----
            acc = spool.tile([128, 1024], DTB, tag="acc")   # max over duos
            nc.vector.memset(acc[:], -1e30)
            acc_s = spool.tile([128, 1024], DTB, tag="acc_s")  # DVE sum part
            nc.vector.memset(acc_s[:], 0.0)
            sum_ps = psump.tile([128, FQ], DT, tag="sum_ps")

            # first/last PE-sum chunk bookkeeping for start/stop flags
            pe_sum_chunks = [
                (t, d) for t in range(NT) for d in range(ND) if not _dve_sum(t, d)
            ]
            first_sum = pe_sum_chunks[0]
            last_sum = pe_sum_chunks[-1]

            # ---- main streaming phase ----
            xs_r = xs[:].rearrange("b (t h g) c -> t b h (g c)", h=2, g=G)

            def emit_sum(t, d, xt_r):
                if _dve_sum(t, d):
                    nc.vector.tensor_add(
                        acc_s[:], acc_s[:], xt_r[:, d * 1024:(d + 1) * 1024]
                    )
                else:
                    for q in range(2):
                        col = d * 1024 + q * FQ
                        nc.tensor.matmul(
                            sum_ps[0:64, :], wstk_sb[:],
                            xt_r[:, col:col + FQ],
                            start=((t, d) == first_sum and q == 0),
                            stop=((t, d) == last_sum and q == 1),
                            skip_group_check=True,
                        )

            for t in range(NT):
                xt = xpool.tile([128, F], DTB, tag="xt")
                nc.sync.dma_start(xt[:], xs_r[t])
                xt_r = xt[:]
                # interleave the two PE passes: the PE queue is in-order, so
                # sum matmuls (always-available PSUM bank) emitted between ym
                # duos fill the stalls while a duo buffer waits to drain
                for d in range(ND):
                    duo = ppool.tile([128, 1024], DT, tag="duo")
                    for q in range(2):
                        col = d * 1024 + q * FQ
                        nc.tensor.matmul(
                            duo[:, q * FQ:(q + 1) * FQ], wblk_sb[:],
                            xt_r[:, col:col + FQ], start=True, stop=True,
                        )
                    if _dve_direct(t, d):
                        nc.vector.tensor_max(acc[:], acc[:], duo[:])
                    else:
                        ymb = ypool.tile([128, 1024], DTB, tag="ymb")
                        nc.scalar.copy(ymb[:], duo[:])
                        nc.vector.tensor_max(acc[:], acc[:], ymb[:])
                    if d >= 1:
                        emit_sum(t, d - 1, xt_r)
                emit_sum(t, ND - 1, xt_r)

            # ---- fold local partials to [64, C] each ----
            # g-fold (free dim): [128, 1024] -> [128, 256]
            nc.vector.tensor_max(acc[:, 0:512], acc[:, 0:512], acc[:, 512:1024])
            nc.vector.tensor_max(acc[:, 0:256], acc[:, 0:256], acc[:, 256:512])
            nc.vector.tensor_add(acc_s[:, 0:512], acc_s[:, 0:512], acc_s[:, 512:1024])
            nc.vector.tensor_add(acc_s[:, 0:256], acc_s[:, 0:256], acc_s[:, 256:512])
            # h-fold (partition pairs 2i, 2i+1) via strided SBUF->SBUF DMA
            hf = spool.tile([128, 4, 256], DTB, tag="hf")
            nc.sync.dma_start(hf[0:64, 0, :], acc[0:128:2, 0:256])
            nc.scalar.dma_start(hf[0:64, 1, :], acc[1:128:2, 0:256])
            nc.sync.dma_start(hf[0:64, 2, :], acc_s[0:128:2, 0:256])
            nc.scalar.dma_start(hf[0:64, 3, :], acc_s[1:128:2, 0:256])
            pk = spool.tile([128, 2 * C], DTB, tag="pk")
            nc.vector.tensor_max(pk[0:64, 0:C], hf[0:64, 0, :], hf[0:64, 1, :])
            # PE sum bank evict + col-half fold + add the DVE sum part
            su = spool.tile([128, FQ], DT, tag="su")
            nc.scalar.copy(su[0:64, :], sum_ps[0:64, :])
            sv = spool.tile([128, C], DT, tag="sv")
            nc.vector.tensor_add(sv[0:64, :], su[0:64, 0:C], su[0:64, C:2 * C])
            nc.vector.tensor_add(sv[0:64, :], sv[0:64, :], hf[0:64, 2, :])
            nc.vector.tensor_add(pk[0:64, C:2 * C], sv[0:64, :], hf[0:64, 3, :])

            # ---- cross-core combine: one bf16 AllGather + local fold ----
            gin = dpool.tile([64, 2 * C], DTB, tag="gin")
            gout = dpool.tile([NCORES * 64, 2 * C], DTB, tag="gout")
            nc.sync.dma_start(gin[:], pk[0:64, :])
            nc.gpsimd.collective_compute(
                "AllGather", mybir.AluOpType.bypass, replica_groups=rg,
                ins=[gin.opt()], outs=[gout.opt()],
            )
            g_sb = spool.tile([128, NCORES, 2 * C], DTB, tag="g_sb")
            nc.scalar.dma_start(
                g_sb[0:64, :, :], gout[:].rearrange("(r b) n -> b r n", r=NCORES)
            )
            mxf = spool.tile([128, C], DT, tag="mxf")
            svf = spool.tile([128, C], DT, tag="svf")
            nc.vector.tensor_max(mxf[0:64, :], g_sb[0:64, 0, 0:C], g_sb[0:64, 1, 0:C])
            nc.vector.tensor_add(
                svf[0:64, :], g_sb[0:64, 0, C:2 * C], g_sb[0:64, 1, C:2 * C]
            )
            for r in range(2, NCORES):
                nc.vector.tensor_max(mxf[0:64, :], mxf[0:64, :], g_sb[0:64, r, 0:C])
                nc.vector.tensor_add(
                    svf[0:64, :], svf[0:64, :], g_sb[0:64, r, C:2 * C]
                )

            # window sums -> window means: scale rows by 1/w_i
            nc.vector.tensor_scalar_mul(mxf[0:64, :], mxf[0:64, :], wsc_sb[0:64, :])
            nc.vector.tensor_scalar_mul(svf[0:64, :], svf[0:64, :], wsc_sb[0:64, :])

            # ---- transpose [64, 256] -> [128, 2, 64] (chunked over C) ----
            def transpose_bc(src, tag):
                dst = spool.tile([128, 2, 64], DTB, tag=tag)
                for ch in range(2):
                    pt = ptail.tile([128, 64], DT, tag="ph")
                    nc.tensor.transpose(
                        pt[:], src[0:64, ch * 128:(ch + 1) * 128], eye_sb[0:64, :]
                    )
                    nc.scalar.copy(dst[:, ch, :], pt[:])
                return dst

            mxT = transpose_bc(mxf, "mxT")
            svT = transpose_bc(svf, "svT")

            # ---- the two tiny MLPs (relu(relu(v @ W1 + b1) @ W2 + b2)) ----
            def mlp(vT, w1_sb, b1_sb, w2_sb, b2_sb, tag):
                h1 = spool.tile([128, 6, 64], DTB, tag=f"h1_{tag}")
                for m in range(6):
                    ph = ptail.tile([128, 64], DT, tag="ph")
                    nc.tensor.matmul(
                        ph[:], w1_sb[:, 0, m * 128:(m + 1) * 128], vT[:, 0, :],
                        start=True, stop=False,
                    )
                    nc.tensor.matmul(
                        ph[:], w1_sb[:, 1, m * 128:(m + 1) * 128], vT[:, 1, :],
                        start=False, stop=True,
                    )
                    nc.scalar.activation(
                        h1[:, m, :], ph[:], AF.Relu, bias=b1_sb[:, m:m + 1]
                    )
                pa = papool.tile([128, C], DT, tag="pa")
                for m in range(6):
                    nc.tensor.matmul(
                        pa[0:64, :], h1[:, m, :], w2_sb[:, m, :],
                        start=(m == 0), stop=False,
                    )
                nc.tensor.matmul(
                    pa[0:64, :], ones_sb, b2_sb, start=False, stop=True
                )
                dst = spool.tile([128, C], DT, tag=f"mlp_{tag}")
                nc.scalar.activation(dst[0:64, :], pa[0:64, :], AF.Relu)
                return dst

            m_sb = mlp(mxT, w1m_sb, b1m_sb, w2m_sb, b2m_sb, "m")
            a_sb = mlp(svT, w1a_sb, b1a_sb, w2a_sb, b2a_sb, "a")

            # ---- sigmoid(m + a) via tanh (same ACT table set as exp):
            #      sigmoid(z) = 0.5 * tanh(0.5 z) + 0.5 ----
            t_sb = spool.tile([128, C], DT, tag="t_sb")
            nc.vector.tensor_add(t_sb[0:64, :], m_sb[0:64, :], a_sb[0:64, :])
            th = spool.tile([128, C], DT, tag="th")
            nc.scalar.activation(th[0:64, :], t_sb[0:64, :], AF.Tanh, scale=0.5)
            s_sb = spool.tile([128, C], DT, tag="s_sb")
            nc.vector.tensor_scalar(
                s_sb[0:64, :], th[0:64, :], 0.5, 0.5,
                op0=mybir.AluOpType.mult, op1=mybir.AluOpType.add,
            )
            # softmax over channels
            red = spool.tile([128, 4], DT, tag="red")
            e_sb = spool.tile([128, C], DT, tag="e_sb")
            nc.scalar.activation(e_sb[0:64, :], s_sb[0:64, :], AF.Exp)
            nc.vector.tensor_reduce(
                red[0:64, 1:2], e_sb[0:64, :], axis=mybir.AxisListType.X,
                op=mybir.AluOpType.add,
            )
            nc.vector.reciprocal(red[0:64, 2:3], red[0:64, 1:2])
            att = spool.tile([128, C], DTB, tag="att")
            nc.vector.tensor_scalar_mul(att[0:64, :], e_sb[0:64, :], red[0:64, 2:3])

            # ---- final cyclic window mean + store ----
            po = papool.tile([128, C], DT, tag="pa")
            nc.tensor.matmul(
                po[0:64, :], wfin_sb[0:64, :], att[0:64, :], start=True, stop=True
            )
            ob = spool.tile([128, C], DT, tag="ob")
            nc.scalar.copy(ob[0:64, :], po[0:64, :])
            nc.sync.dma_start(out_d[:], ob[0:64, :])

    return nc


def run(inputs: dict, trace: bool = False, tmpdir: str | None = None):
    """Returns (full_output [64,256] f32, exec_time_ns or None)."""
    wn = int(np.asarray(inputs["windows"]))
    x = np.ascontiguousarray(np.asarray(inputs["x"], np.float32)).reshape(B, S, C)

    import ml_dtypes
    mwin = _win_matrix(wn)
    # 0/1 window-membership matrix (exact in bf16); per-row counts -> wsc
    m01 = (mwin > 0).astype(np.float32).T           # [j, i]
    cnt = (mwin > 0).sum(axis=1).astype(np.float32)  # rows of M
    wsc = (1.0 / cnt).reshape(64, 1)
    # partition packing p = 2*b + h: wblk[2j+h, 2i+h'] = m01[j,i] iff h==h'
    wblk = np.zeros((128, 128), np.float32)
    wstk = np.zeros((128, 64), np.float32)
    for h in range(2):
        wblk[h::2, h::2] = m01
        wstk[h::2, :] = m01
    bf = ml_dtypes.bfloat16
    eye = np.eye(64, dtype=np.float32)

    common = {
        "wblk": wblk.astype(bf),
        "wstk": np.ascontiguousarray(wstk).astype(bf),
        "wsc": wsc.astype(np.float32),
        "wfin": np.ascontiguousarray(mwin.T).astype(bf),
        "eye64": eye,
        "w1m": np.asarray(inputs["W1_max"], np.float32).astype(bf),
        "b1m": np.asarray(inputs["b1_max"], np.float32).reshape(CE),
        "w2m": np.asarray(inputs["W2_max"], np.float32).astype(bf),
        "b2m": np.asarray(inputs["b2_max"], np.float32).reshape(1, C),
        # avg branch consumes the raw spatial SUM; fold the 1/S into W1_avg
        "w1a": (np.asarray(inputs["W1_avg"], np.float32) / np.float32(S)).astype(bf),
        "b1a": np.asarray(inputs["b1_avg"], np.float32).reshape(CE),
        "w2a": np.asarray(inputs["W2_avg"], np.float32).astype(bf),
        "b2a": np.asarray(inputs["b2_avg"], np.float32).reshape(1, C),
    }
    in_maps = []
    for k in range(NCORES):
        m = dict(common)
        m["xs"] = np.ascontiguousarray(
            x[:, k * S_CORE:(k + 1) * S_CORE, :]
        ).astype(bf)
        in_maps.append(m)

    nc = _build_program(wn)
    nc.compile()
    res = run_bass_kernel_spmd(
        nc, in_maps, list(range(NCORES)), trace=trace, tmpdir=tmpdir,
    )
    out = np.asarray(res.results[0]["out"], np.float32)
    return out, res.exec_time_ns


def kernel(**inputs) -> np.ndarray:
    out, _ = run(inputs, trace=False)
    return out


# revision 15
# speedup vs baseline: 1.1901x; 1.1175x over previous
"""Trainium2 Bass kernel for nn_ChannelAttentionModule (cyclic window mean +
channel attention). Self-contained: accepts FULL inputs, shards the CHANNEL
dim across 8 NeuronCores, returns FULL [64, 256] output.

Math: cyclic_window_mean over the batch axis is a matmul with a fixed [64,64]
window matrix M.  Per core (channel shard: 32 of the 256 channels, full
spatial):
  - stream x as [128, 8192] bf16 tiles (partition p = 2b + h, h = spatial
    half-group; free = (g, c) with g = 256 positions, c = 32 channels)
  - PE pass 1: blockdiag window stationary -> per-position window SUMS into
    PSUM duos [128, 1024] (one N=1024 bf16 matmul each)
  - drain: 7 of 8 duos ACT-evicted to bf16 SBUF then DVE max-accumulated in
    2x mode; 1 of 8 DVE-maxed straight from PSUM (engine balance)
  - spatial-SUM pass split by quota: most duos as wstk matmuls accumulating
    into one PSUM bank, the rest as DVE / GPSIMD bf16 adds on the resident
    x tiles
  - fold h/g locally, scale by 1/w_i, TRANSPOSE the [64, 64] partial ->
    tiny bf16 AllGather [64,64]/rank -> gathered result IS the transposed
    MLP input; every core runs the MLP / softmax / final window matmul
    redundantly.
"""

import os
import sys

import numpy as np

for _p in ("/opt/trn_rl_repo", "/root/.axon_site/_ro/trn_rl_repo"):
    if os.path.isdir(_p) and _p not in sys.path:
        sys.path.insert(0, _p)

import concourse.bass as bass
import concourse.mybir as mybir
import concourse.tile as tile
from concourse import bacc
from concourse.bass_utils import run_bass_kernel_spmd

B = 64          # batch
S = 64 * 64     # flattened spatial
C = 256         # channels
CE = 768        # hidden (C * 3)
NCORES = 8
C_CORE = C // NCORES   # 32 channels per core
G = 256                # positions per partition half-group per tile
TP = 2 * G             # 512 positions per tile
NT = S // TP           # 8 tiles per core
F = G * C_CORE         # 8192 free elements per tile
FQ = 512
ND = F // 1024         # 8 PSUM duos [128, 1024] per tile
DT = mybir.dt.float32
DTB = mybir.dt.bfloat16
AF = mybir.ActivationFunctionType

# per-duo routing quotas (see kernel notes): max-drain route and sum route
# max: duos 0..6 ACT-evict + DVE bf16 max; duo 7 DVE direct from PSUM
MAX_DVE_DIRECT = {7}
# sum route per (t, d): 'p' = PE wstk matmuls, 'v' = DVE add, 'g' = GPSIMD add
def _sum_route(t: int, d: int) -> str:
    if d == 7 or (d == 6 and t < 4):
        return "g"          # 12 duos
    if d == 5 or (d == 6 and t >= 4):
        return "v"          # 12 duos
    return "p"              # 40 duos


def _win_matrix(w: int) -> np.ndarray:
    """M such that cyclic_window_mean(x, w) == M @ x (along axis 0)."""
    m = np.zeros((B, B), np.float64)
    for i in range(B):
        if i >= w:
            m[i, i - w:i] = 1.0 / w
        else:
            m[i, : i + 1] = 1.0 / (w + 1)
            m[i, B - (w - i):] = 1.0 / (w + 1)
    return m.astype(np.float32)


def _build_program(wn: int) -> bass.Bass:
    nc = bacc.Bacc(
        "TRN2", target_bir_lowering=False, debug=False, num_devices=NCORES
    )

    xs = nc.declare_dram_parameter("xs", [B, S, C_CORE], DTB, isOutput=False)
    wblk_d = nc.declare_dram_parameter("wblk", [128, 128], DTB, isOutput=False)
    wstk_d = nc.declare_dram_parameter("wstk", [128, 64], DTB, isOutput=False)
    wsc_d = nc.declare_dram_parameter("wsc", [64, 1], DT, isOutput=False)
    wfin_d = nc.declare_dram_parameter("wfin", [64, 64], DTB, isOutput=False)
    eye_d = nc.declare_dram_parameter("eye64", [64, 64], DT, isOutput=False)
    w1m_d = nc.declare_dram_parameter("w1m", [C, CE], DTB, isOutput=False)
    b1m_d = nc.declare_dram_parameter("b1m", [CE], DT, isOutput=False)
    w2m_d = nc.declare_dram_parameter("w2m", [CE, C], DTB, isOutput=False)
    b2m_d = nc.declare_dram_parameter("b2m", [1, C], DT, isOutput=False)
    w1a_d = nc.declare_dram_parameter("w1a", [C, CE], DTB, isOutput=False)
    b1a_d = nc.declare_dram_parameter("b1a", [CE], DT, isOutput=False)
    w2a_d = nc.declare_dram_parameter("w2a", [CE, C], DTB, isOutput=False)
    b2a_d = nc.declare_dram_parameter("b2a", [1, C], DT, isOutput=False)
    out_d = nc.declare_dram_parameter("out", [B, C], DT, isOutput=True)

    with tile.TileContext(nc) as tc:
        with (
            tc.tile_pool(name="const", bufs=1) as cpool,
            tc.tile_pool(name="x", bufs=4) as xpool,
            tc.tile_pool(name="ymb", bufs=4) as ypool,
            tc.tile_pool(name="pduo", bufs=2, space="PSUM") as ppool,
            tc.tile_pool(name="psum_sum", bufs=1, space="PSUM") as psump,
            tc.tile_pool(name="ptail", bufs=2, space="PSUM") as ptail,
            tc.tile_pool(name="pa_pool", bufs=1, space="PSUM") as papool,
            tc.tile_pool(name="sb", bufs=1) as spool,
            tc.tile_pool(name="dram", bufs=1, space="DRAM") as dpool,
        ):
            rg = [list(range(NCORES))]

            # ---- warmup collective: sync the ranks / CC control plane early
            #      so the real AllGather doesn't eat the barrier cost ----
            wrm_in = dpool.tile([8, 16], DT, tag="wrm_in")
            wrm_out = dpool.tile([NCORES * 8, 16], DT, tag="wrm_out")
            wrm_sb = cpool.tile([8, 16], DT, tag="wrm_sb")
            nc.vector.memset(wrm_sb[:], 0.0)
            nc.sync.dma_start(wrm_in[:], wrm_sb[:])
            nc.gpsimd.collective_compute(
                "AllGather", mybir.AluOpType.bypass, replica_groups=rg,
                ins=[wrm_in.opt()], outs=[wrm_out.opt()],
            )

            # ---- streaming-phase constants (everything else loads late) ----
            wblk_sb = cpool.tile([128, 128], DTB, tag="wblk")
            nc.sync.dma_start(wblk_sb[:], wblk_d[:])
            wstk_sb = cpool.tile([128, 64], DTB, tag="wstk")
            nc.sync.dma_start(wstk_sb[:], wstk_d[:])

            # preload the exp_and_others ACT table set (exp + tanh + relu +
            # copy) so no table switch lands on the critical path later
            scratch = cpool.tile([128, 16], DT, tag="scratch")
            nc.vector.memset(scratch[0:1, :], 0.0)
            nc.scalar.activation(scratch[0:1, :], scratch[0:1, :], AF.Exp)

            # ---- accumulators ----
            acc = spool.tile([128, 1024], DTB, tag="acc")      # window-sum max
            nc.vector.memset(acc[:], -1e30)
            acc_sv = spool.tile([128, 1024], DTB, tag="acc_sv")  # DVE sum part
            nc.vector.memset(acc_sv[:], 0.0)
            acc_sg = spool.tile([128, 1024], DTB, tag="acc_sg")  # GPSIMD part
            nc.gpsimd.memset(acc_sg[:], 0.0)
            sum_ps = psump.tile([128, FQ], DT, tag="sum_ps")

            pe_sum_chunks = [
                (t, d) for t in range(NT) for d in range(ND)
                if _sum_route(t, d) == "p"
            ]
            first_sum = pe_sum_chunks[0]
            last_sum = pe_sum_chunks[-1]

            # ---- main streaming phase ----
            xs_r = xs[:].rearrange("b (t h g) c -> t b h (g c)", h=2, g=G)

            def emit_sum(t, d, xt_r):
                route = _sum_route(t, d)
                lo, hi = d * 1024, (d + 1) * 1024
                if route == "v":
                    nc.vector.tensor_add(acc_sv[:], acc_sv[:], xt_r[:, lo:hi])
                elif route == "g":
                    nc.gpsimd.tensor_add(acc_sg[:], acc_sg[:], xt_r[:, lo:hi])
                else:
                    for q in range(2):
                        col = lo + q * FQ
                        nc.tensor.matmul(
                            sum_ps[0:64, :], wstk_sb[:],
                            xt_r[:, col:col + FQ],
                            start=((t, d) == first_sum and q == 0),
                            stop=((t, d) == last_sum and q == 1),
                            skip_group_check=True,
                        )

            for t in range(NT):
                xt = xpool.tile([128, F], DTB, tag="xt")
                nc.sync.dma_start(xt[:, 0:F // 2], xs_r[t, :, :, 0:F // 2])
                nc.sync.dma_start(xt[:, F // 2:F], xs_r[t, :, :, F // 2:F])
                xt_r = xt[:]
                # interleave the passes: PE queue is in-order, so sum matmuls
                # emitted between ym duos fill the duo-drain stalls
                for d in range(ND):
                    duo = ppool.tile([128, 1024], DT, tag="duo")
                    if os.environ.get("MM1024", "1") == "1":
                        nc.tensor.matmul(
                            duo[:], wblk_sb[:], xt_r[:, d * 1024:(d + 1) * 1024],
                            start=True, stop=True,
                        )
                    else:
                        for q in range(2):
                            col = d * 1024 + q * FQ
                            nc.tensor.matmul(
                                duo[:, q * FQ:(q + 1) * FQ], wblk_sb[:],
                                xt_r[:, col:col + FQ], start=True, stop=True,
                            )
                    if d in MAX_DVE_DIRECT:
                        nc.vector.tensor_max(acc[:], acc[:], duo[:])
                    else:
                        ymb = ypool.tile([128, 1024], DTB, tag="ymb")
                        nc.scalar.copy(ymb[:], duo[:])
                        nc.vector.tensor_max(acc[:], acc[:], ymb[:])
                    if d >= 1:
                        emit_sum(t, d - 1, xt_r)
                emit_sum(t, ND - 1, xt_r)

            # ---- tail-phase constants (loaded during the stream tail) ----
            wfin_sb = cpool.tile([128, 64], DTB, tag="wfin")
            nc.gpsimd.dma_start(wfin_sb[0:64, :], wfin_d[:])
            wsc_sb = cpool.tile([128, 1], DT, tag="wsc")
            nc.gpsimd.dma_start(wsc_sb[0:64, :], wsc_d[:])
            eye_sb = cpool.tile([128, 64], DT, tag="eye")
            nc.gpsimd.dma_start(eye_sb[0:64, :], eye_d[:])
            w1m_sb = cpool.tile([128, 2, CE], DTB, tag="w1m")
            nc.gpsimd.dma_start(w1m_sb[:], w1m_d[:].rearrange("(h k) n -> k h n", h=2))
            w1a_sb = cpool.tile([128, 2, CE], DTB, tag="w1a")
            nc.gpsimd.dma_start(w1a_sb[:], w1a_d[:].rearrange("(h k) n -> k h n", h=2))
            w2m_sb = cpool.tile([128, 6, C], DTB, tag="w2m")
            nc.gpsimd.dma_start(w2m_sb[:], w2m_d[:].rearrange("(m k) n -> k m n", m=6))
            w2a_sb = cpool.tile([128, 6, C], DTB, tag="w2a")
            nc.gpsimd.dma_start(w2a_sb[:], w2a_d[:].rearrange("(m k) n -> k m n", m=6))
            b1m_sb = cpool.tile([128, 6], DT, tag="b1m")
            nc.gpsimd.dma_start(b1m_sb[:], b1m_d[:].rearrange("(m k) -> k m", m=6))
            b1a_sb = cpool.tile([128, 6], DT, tag="b1a")
            nc.gpsimd.dma_start(b1a_sb[:], b1a_d[:].rearrange("(m k) -> k m", m=6))
            smalls = cpool.tile([128, 576], DT, tag="smalls")
            nc.vector.memset(smalls[0:1, 0:64], 1.0)
            nc.gpsimd.dma_start(smalls[0:1, 64:320], b2m_d[:])
            nc.gpsimd.dma_start(smalls[0:1, 320:576], b2a_d[:])
            ones_sb = smalls[0:1, 0:64]
            b2m_sb = smalls[0:1, 64:320]
            b2a_sb = smalls[0:1, 320:576]

            # ---- fold local partials to [64, C_CORE] each ----
            # h-fold (partition pairs 2i, 2i+1) via strided SBUF->SBUF DMA
            hf = spool.tile([128, 4, 1024], DTB, tag="hf")
            nc.vector.tensor_add(acc_sv[:], acc_sv[:], acc_sg[:])
            nc.sync.dma_start(hf[0:64, 0, :], acc[0:128:2, :])
            nc.sync.dma_start(hf[0:64, 1, :], acc[1:128:2, :])
            nc.sync.dma_start(hf[0:64, 2, :], acc_sv[0:128:2, :])
            nc.sync.dma_start(hf[0:64, 3, :], acc_sv[1:128:2, :])
            um = spool.tile([128, 1024], DTB, tag="um")
            nc.vector.tensor_max(um[0:64, :], hf[0:64, 0, :], hf[0:64, 1, :])
            us = spool.tile([128, 1024], DTB, tag="us")
            nc.vector.tensor_add(us[0:64, :], hf[0:64, 2, :], hf[0:64, 3, :])
            # g-fold (free dim): 1024 = (g 32, c 32) -> 32
            for w in (512, 256, 128, 64):
                nc.vector.tensor_max(um[0:64, 0:w], um[0:64, 0:w], um[0:64, w:2 * w])
                nc.vector.tensor_add(us[0:64, 0:w], us[0:64, 0:w], us[0:64, w:2 * w])
            # PE sum bank: evict + g-fold 512 = (g 16, c 32) -> 32
            su = spool.tile([128, FQ], DT, tag="su")
            nc.scalar.copy(su[0:64, :], sum_ps[0:64, :])
            for w in (256, 128, 64):
                nc.vector.tensor_add(su[0:64, 0:w], su[0:64, 0:w], su[0:64, w:2 * w])
            # combine, scale rows by 1/w_i, pack [64, 64] with interleaved
            # cols (c, t): col = 2c + t, t=0 max / t=1 sum -- so the
            # AllGather output rearrange below has adjacent grouped dims
            pre = spool.tile([128, 64], DT, tag="pre")
            nc.vector.tensor_max(pre[0:64, 0:64:2], um[0:64, 0:32], um[0:64, 32:64])
            nc.vector.tensor_add(su[0:64, 0:32], su[0:64, 0:32], su[0:64, 32:64])
            nc.vector.tensor_add(pre[0:64, 1:64:2], su[0:64, 0:32], us[0:64, 0:32])
            nc.vector.tensor_scalar_mul(pre[0:64, :], pre[0:64, :], wsc_sb[0:64, :])
            # transpose -> [64 rows = (t=2, c=32), 64 b] so the AllGather
            # output is directly the transposed MLP input
            ptp = ptail.tile([128, 64], DT, tag="ph")
            nc.tensor.transpose(ptp[0:64, :], pre[0:64, :], eye_sb[0:64, :])
            pkT = spool.tile([128, 64], DTB, tag="pkT")
            nc.scalar.copy(pkT[0:64, :], ptp[0:64, :])

            # ---- cross-core combine: tiny bf16 AllGather, no post-fold ----
            gin = dpool.tile([64, 64], DTB, tag="gin")
            gout = dpool.tile([NCORES * 64, 64], DTB, tag="gout")
            nc.sync.dma_start(gin[:], pkT[0:64, :])
            nc.gpsimd.collective_compute(
                "AllGather", mybir.AluOpType.bypass, replica_groups=rg,
                ins=[gin.opt()], outs=[gout.opt()],
            )
            # gout rows = (r, c, t) with channel c' = 32r + c = 128h + k:
            # r = 4h + ro, k = 32 ro + c
            g_r = gout[:].rearrange(
                "(h ro c t) b -> t (ro c) h b", h=2, ro=4, c=32, t=2
            )
            mxT = spool.tile([128, 2, 64], DTB, tag="mxT")
            nc.sync.dma_start(mxT[:], g_r[0])
            svT = spool.tile([128, 2, 64], DTB, tag="svT")
            nc.sync.dma_start(svT[:], g_r[1])

            # ---- the two tiny MLPs (relu(relu(v @ W1 + b1) @ W2 + b2)) ----
            def mlp(vT, w1_sb, b1_sb, w2_sb, b2_sb, tag):
                h1 = spool.tile([128, 6, 64], DTB, tag=f"h1_{tag}")
                for m in range(6):
                    ph = ptail.tile([128, 64], DT, tag="ph")
                    nc.tensor.matmul(
                        ph[:], w1_sb[:, 0, m * 128:(m + 1) * 128], vT[:, 0, :],
                        start=True, stop=False,
                    )
                    nc.tensor.matmul(
                        ph[:], w1_sb[:, 1, m * 128:(m + 1) * 128], vT[:, 1, :],
                        start=False, stop=True,
                    )
                    nc.scalar.activation(
                        h1[:, m, :], ph[:], AF.Relu, bias=b1_sb[:, m:m + 1]
                    )
                pa = papool.tile([128, C], DT, tag="pa")
                for m in range(6):
                    nc.tensor.matmul(
                        pa[0:64, :], h1[:, m, :], w2_sb[:, m, :],
                        start=(m == 0), stop=False,
                    )
                nc.tensor.matmul(
                    pa[0:64, :], ones_sb, b2_sb, start=False, stop=True
                )
                dst = spool.tile([128, C], DT, tag=f"mlp_{tag}")
                nc.scalar.activation(dst[0:64, :], pa[0:64, :], AF.Relu)
                return dst

            m_sb = mlp(mxT, w1m_sb, b1m_sb, w2m_sb, b2m_sb, "m")
            a_sb = mlp(svT, w1a_sb, b1a_sb, w2a_sb, b2a_sb, "a")

            # ---- sigmoid(m + a) via tanh (same ACT table set as exp):
            #      sigmoid(z) = 0.5 * tanh(0.5 z) + 0.5 ----
            t_sb = spool.tile([128, C], DT, tag="t_sb")
            nc.vector.tensor_add(t_sb[0:64, :], m_sb[0:64, :], a_sb[0:64, :])
            th = spool.tile([128, C], DT, tag="th")
            nc.scalar.activation(th[0:64, :], t_sb[0:64, :], AF.Tanh, scale=0.5)
            s_sb = spool.tile([128, C], DT, tag="s_sb")
            nc.vector.tensor_scalar(
                s_sb[0:64, :], th[0:64, :], 0.5, 0.5,
                op0=mybir.AluOpType.mult, op1=mybir.AluOpType.add,
            )
            # softmax over channels
            red = spool.tile([128, 4], DT, tag="red")
            e_sb = spool.tile([128, C], DT, tag="e_sb")
            nc.scalar.activation(e_sb[0:64, :], s_sb[0:64, :], AF.Exp)
            nc.vector.tensor_reduce(
                red[0:64, 1:2], e_sb[0:64, :], axis=mybir.AxisListType.X,
                op=mybir.AluOpType.add,
            )
            nc.vector.reciprocal(red[0:64, 2:3], red[0:64, 1:2])
            att = spool.tile([128, C], DTB, tag="att")
            nc.vector.tensor_scalar_mul(att[0:64, :], e_sb[0:64, :], red[0:64, 2:3])

            # ---- final cyclic window mean + store ----
            po = papool.tile([128, C], DT, tag="pa")
            nc.tensor.matmul(
                po[0:64, :], wfin_sb[0:64, :], att[0:64, :], start=True, stop=True
            )
            ob = spool.tile([128, C], DT, tag="ob")
            nc.scalar.copy(ob[0:64, :], po[0:64, :])
            nc.sync.dma_start(out_d[:], ob[0:64, :])

    return nc


def run(inputs: dict, trace: bool = False, tmpdir: str | None = None):
    """Returns (full_output [64,256] f32, exec_time_ns or None)."""
    wn = int(np.asarray(inputs["windows"]))
    x = np.ascontiguousarray(np.asarray(inputs["x"], np.float32)).reshape(B, S, C)

    import ml_dtypes
    mwin = _win_matrix(wn)
    # 0/1 window-membership matrix (exact in bf16); per-row counts -> wsc
    m01 = (mwin > 0).astype(np.float32).T           # [j, i]
    cnt = (mwin > 0).sum(axis=1).astype(np.float32)  # rows of M
    wsc = (1.0 / cnt).reshape(64, 1)
    # partition packing p = 2*b + h: wblk[2j+h, 2i+h'] = m01[j,i] iff h==h'
    wblk = np.zeros((128, 128), np.float32)
    wstk = np.zeros((128, 64), np.float32)
    for h in range(2):
        wblk[h::2, h::2] = m01
        wstk[h::2, :] = m01
    bf = ml_dtypes.bfloat16
    eye = np.eye(64, dtype=np.float32)

    common = {
        "wblk": wblk.astype(bf),
        "wstk": np.ascontiguousarray(wstk).astype(bf),
        "wsc": wsc.astype(np.float32),
        "wfin": np.ascontiguousarray(mwin.T).astype(bf),
        "eye64": eye,
        "w1m": np.asarray(inputs["W1_max"], np.float32).astype(bf),
        "b1m": np.asarray(inputs["b1_max"], np.float32).reshape(CE),
        "w2m": np.asarray(inputs["W2_max"], np.float32).astype(bf),
        "b2m": np.asarray(inputs["b2_max"], np.float32).reshape(1, C),
        # avg branch consumes the raw spatial SUM; fold the 1/S into W1_avg
        "w1a": (np.asarray(inputs["W1_avg"], np.float32) / np.float32(S)).astype(bf),
        "b1a": np.asarray(inputs["b1_avg"], np.float32).reshape(CE),
        "w2a": np.asarray(inputs["W2_avg"], np.float32).astype(bf),
        "b2a": np.asarray(inputs["b2_avg"], np.float32).reshape(1, C),
    }
    in_maps = []
    for k in range(NCORES):
        m = dict(common)
        m["xs"] = np.ascontiguousarray(
            x[:, :, k * C_CORE:(k + 1) * C_CORE]
        ).astype(bf)
        in_maps.append(m)

    nc = _build_program(wn)
    nc.compile()
    res = run_bass_kernel_spmd(
        nc, in_maps, list(range(NCORES)), trace=trace, tmpdir=tmpdir,
    )
    out = np.asarray(res.results[0]["out"], np.float32)
    return out, res.exec_time_ns


def kernel(**inputs) -> np.ndarray:
    out, _ = run(inputs, trace=False)
    return out
